# revision 73
# baseline (speedup 1.0000x reference)
"""Trainium2 Bass kernel: Based linear attention (poly feature map, causal, normalized).

Full inputs q,k,v: [1, 16, 4096, 16] fp32. Output: [1, 16, 4096, 16] fp32.
Sharding: 16 heads over 8 cores (2 heads/core); each head is independent.
Host side permutes IO to [128, head, chunk, d] so every DMA moves
contiguous >=512B lines at full bandwidth.

Algorithm (per head): chunked quadratic-state linear attention, C=128.
  P = 1 + s + 0.5 s^2 with s = u.k, u = q/sqrt(D).
  Intra chunk: stp[j,i] = [1|k_j].[1|u_i] = 1+s (PE, 17-feature contraction
  from DMA-xbar-transposed tiles); P = Square(stp/sqrt2) masked (j<=i) plus
  0.5-masked (trih matmul). Cross chunk: explicit quadratic features built
  in 4-chunk groups, q2[i,(p,r)] = u_p u_r (DVE) padded to 256 cols/chunk
  and transposed to [f,i] layout by one DMA xbar pass per (head, group);
  k2[j,(p,r)] = w_pr k_p k_r (Pool) carries the symmetry weights via
  pre-scaled kw/khh operands.  States M2[f,d'] = sum k2^T v',
  M1[e,d'] = sum [1|k]^T v' accumulate in PSUM; numerator = intra matmuls +
  q2t^T @ M2snap + [1|u]^T-read of M1snap accumulated in an 8-chunk PSUM
  num tile (v' = [v|1] carries the normalizer z in channel 16).  Snapshots
  on Act (DVE for the tail chunks where DVE is idle).  Normalization reads
  num straight from PSUM every 8 chunks and stores contiguously.
  Front-end runs LAG=5 chunks ahead of the state back-end to cover the
  q2 xbar latency.
"""
import numpy as np
from contextlib import ExitStack

import concourse.bass as bass
import concourse.bacc as bacc
import concourse.tile as tile
import concourse.mybir as mybir
from bass_rust import add_dep_helper
from concourse.masks import make_upper_triangular
from concourse.bass_utils import run_bass_kernel_spmd

B, H, S, D = 1, 16, 4096, 16
NCORES = 8
HPC = H // NCORES  # heads per core (2)
C = 128            # chunk (positions)
NCH = S // C       # 32 chunks
D1 = D + 1         # 17
F2 = 256           # quadratic features (p,r)
FP = 32            # padded feature width for xbar transpose (NCH*FP % 128 == 0, FP % 16 == 0)
dt = mybir.dt.float32
bt = mybir.dt.bfloat16
SCALE = 1.0 / np.sqrt(D)
RT2I = 1.0 / np.sqrt(2.0)
Alu = mybir.AluOpType
Act = mybir.ActivationFunctionType


def _ap(base_ap, offset_ap, dims):
    """AP on the same tensor as `base_ap`: partition dim kept, free dims replaced."""
    return bass.AP(tensor=base_ap.tensor, offset=offset_ap.offset,
                   ap=[base_ap.ap[0]] + dims)


def _build_core(nc, pools, q_d, k_d, v_d, o_d):
    (ident, trih, mask), bulk, sb, sbb, snapp, ps_stp, ps_num, ps_state, ps_kt = pools

    # ---- raw loads (both heads): [h, S, D] -> [128, h, NCH, D] ----
    qraw = bulk.tile([128, HPC, NCH, D], dt, tag="qraw")
    kraw = bulk.tile([128, HPC, NCH, D], dt, tag="kraw")
    vraw = bulk.tile([128, HPC, NCH, D], dt, tag="vraw")
    NQ = NCH // 2

    # loads/prep/xbar are interleaved below (per half)
    # ---- feature tensors ----
    # kb = [1|k] (17 used, padded to 20 for the DMA xbar transpose), ab = [1|u]
    kb = bulk.tile([128, HPC, NCH, FP], bt, tag="kb")
    ab = bulk.tile([128, HPC, NCH, FP], bt, tag="ab")
    vb = bulk.tile([128, HPC, NCH, D1], bt, tag="vb")
    # k-side sym-weight copies: kw = w_x*k_x (0.5 for x<8 else 1), khh = 0.5*k[8:16]
    kw = bulk.tile([128, HPC, NCH, D], bt, tag="kw")
    khh = bulk.tile([128, HPC, NCH, 8], bt, tag="khh")
    warm = bulk.tile([1, 1], dt, tag="actwarm")
    nc.vector.memset(warm[:], 1.0)
    nc.scalar.activation(warm[:], warm[:], Act.Square)
    nc.vector.memset(kb[:, :, :, 0:1], 1.0)
    nc.vector.memset(ab[:, :, :, 0:1], 1.0)
    nc.vector.memset(vb[:, :, :, D : D + 1], 1.0)

    # real xbar layout: out[r, g, j] = in[j, 128*g + r]; with FP=32 each
    # 128-col group g packs 4 chunks at partition bases {0,32,64,96}.
    ktp = bulk.tile([128, HPC, NCH // 4, 128], bt, tag="ktp")
    atp = bulk.tile([128, HPC, NCH // 4, 128], bt, tag="atp")
    NQ4 = NCH // 2
    NG4 = NCH // 2 // 4  # xbar groups per load half

    def emit_load(qt):
        cs = slice(qt * NQ4, (qt + 1) * NQ4)
        nc.sync.dma_start(kraw[:, :, cs], k_d[:, :, cs])
        nc.sync.dma_start(qraw[:, :, cs], q_d[:, :, cs])

    def emit_vload(qt):
        cs = slice(qt * NQ4, (qt + 1) * NQ4)
        nc.sync.dma_start(vraw[:, :, cs], v_d[:, :, cs])

    def emit_prep(qt, c_lo=None, c_hi=None, g_lo=None, g_hi=None):
        c_lo = qt * NQ4 if c_lo is None else c_lo
        c_hi = (qt + 1) * NQ4 if c_hi is None else c_hi
        g_lo = c_lo // 4 if g_lo is None else g_lo
        g_hi = c_hi // 4 if g_hi is None else g_hi
        cs = slice(c_lo, c_hi)
        nc.scalar.copy(kb[:, :, cs, 1 : D + 1], kraw[:, :, cs])
        nc.scalar.mul(ab[:, :, cs, 1 : D + 1], qraw[:, :, cs], SCALE)
        if g_hi > g_lo:
            gs = slice(g_lo, g_hi)
            xcs = slice(g_lo * 4, g_hi * 4)
            for h in range(HPC):
                nc.sync.dma_start_transpose(ktp[:, h, gs], kb[:, h, xcs])
                nc.sync.dma_start_transpose(atp[:, h, gs], ab[:, h, xcs])

    def emit_pe_transp(g):
        # PE transposes for group g (4 chunks) into ktp/atp, 2 batched copies
        tp = ps_kt.tile([128, HPC, 2, 128], bt, tag="kt_ps")
        for h in range(HPC):
            kb_slab = _ap(kb[:], kb[:, h, 4 * g, 0:1], [[1, 128]])
            ab_slab = _ap(ab[:], ab[:, h, 4 * g, 0:1], [[1, 128]])
            nc.tensor.matmul(tp[:, h, 0, :], kb_slab, ident[:], start=True,
                             stop=True, skip_group_check=True, is_transpose=True)
            nc.tensor.matmul(tp[:, h, 1, :], ab_slab, ident[:], start=True,
                             stop=True, skip_group_check=True, is_transpose=True)
        gstr = (NCH // 4) * 128
        kdst = _ap(ktp[:], ktp[:, 0, g, 0:1], [[gstr, HPC], [1, 128]])
        ksrc = _ap(tp[:], tp[:], [[256, HPC], [1, 128]])
        nc.vector.tensor_copy(kdst, ksrc)
        adst = _ap(atp[:], atp[:, 0, g, 0:1], [[gstr, HPC], [1, 128]])
        asrc = _ap(tp[:], tp[:, 0, 1, 0:1], [[256, HPC], [1, 128]])
        nc.vector.tensor_copy(adst, asrc)

    def emit_vprep(qt):
        cs = slice(qt * NQ4, (qt + 1) * NQ4)
        nc.scalar.copy(vb[:, :, cs, 0:D], vraw[:, :, cs])

    def emit_dbl(qt, dve=False):
        def _mul(o, i, s):
            if dve:
                nc.vector.tensor_scalar_mul(o, i, s)
            else:
                nc.scalar.mul(o, i, s)
        c0 = qt * NQ4
        for h in range(HPC):
            klo = _ap(kraw[:], kraw[:, h, c0, 0:1], [[D, NQ4], [1, 8]])
            klo_o = _ap(kw[:], kw[:, h, c0, 0:1], [[D, NQ4], [1, 8]])
            _mul(klo_o, klo, 0.5)
            khi = _ap(kraw[:], kraw[:, h, c0, 8:9], [[D, NQ4], [1, 8]])
            khi_o = _ap(kw[:], kw[:, h, c0, 8:9], [[D, NQ4], [1, 8]])
            _mul(khi_o, khi, 1.0)
            kh2 = _ap(kraw[:], kraw[:, h, c0, 8:9], [[D, NQ4], [1, 8]])
            kh2_o = _ap(khh[:], khh[:, h, c0, 0:1], [[8, NQ4], [1, 8]])
            _mul(kh2_o, kh2, 0.5)

    # all loads issued first (independent, no SP blocking); chunks 0-7
    # prepped + PE-transposed (fast path); groups 2-3 via xbar; half-1
    # prep mid-loop.
    emit_load(0)
    emit_vload(0)
    emit_load(1)
    emit_vload(1)
    emit_prep(0, c_lo=0, c_hi=8, g_lo=0, g_hi=0)
    emit_pe_transp(0)
    emit_pe_transp(1)
    emit_dbl(0)
    emit_vprep(0)
    emit_prep(0, c_lo=8, c_hi=16, g_lo=2, g_hi=4)

    o_sb = bulk.tile([128, HPC, NCH, D], dt, tag="osb")

    # persistent PSUM state: [128, h, 3, 17] = (M2a, M2b, M1)
    st = ps_state.tile([128, HPC, 3, D1], dt, tag="st")
    nc.vector.memset(st[:], 0.0)

    snap_prev = None
    prev_snap_op = None
    cdata = {}

    # software pipeline: at iteration c, issue front-end for chunk c
    # (builds, transposes, scores, square, mask, q2t copy) and back-end for
    # chunk c-1 (readout matmuls, state update, snapshot, tot copy).
    LAG = 6
    for c in range(NCH + LAG):
        if c == 4:
            emit_prep(1)
        if c == 5:
            emit_dbl(1)
        if c == 7:
            emit_vprep(1)
        if c < NCH:
            # --- quadratic feature builds, 4-chunk groups; q2b padded to
            # 256 cols/chunk so one DMA xbar transposes the whole slab:
            # chunk cc blk0 -> xbar group 2cc, blk1 -> group 2cc+1 rows 0:64.
            # Sym weights ride the k side (kw, khh).
            if c % 4 == 0:
                q2b = sbb.tile([128, HPC, 4, 256], bt, tag="q2b")
                k2b = sbb.tile([128, HPC, 4, 192], bt, tag="k2b")
                q2tx = sbb.tile([128, HPC, 8, 128], bt, tag="q2tx")
                for h in range(HPC):
                    a0 = _ap(ab[:], ab[:, h, c, 1:2], [[FP, 4], [0, D], [1, 8]])
                    a1 = _ap(ab[:], ab[:, h, c, 1:2], [[FP, 4], [1, D], [0, 8]])
                    nc.vector.tensor_mul(
                        _ap(q2b[:], q2b[:, h, 0, 0:1], [[256, 4], [1, 128]]),
                        a0, a1)
                    a2 = _ap(ab[:], ab[:, h, c, 9:10], [[FP, 4], [1, 8], [0, 8]])
                    a3 = _ap(ab[:], ab[:, h, c, 9:10], [[FP, 4], [0, 8], [1, 8]])
                    nc.vector.tensor_mul(
                        _ap(q2b[:], q2b[:, h, 0, 128:129], [[256, 4], [1, 64]]),
                        a2, a3)
                    kk0 = _ap(kb[:], kb[:, h, c, 1:2], [[FP, 4], [0, D], [1, 8]])
                    kk1 = _ap(kw[:], kw[:, h, c, 0:1], [[D, 4], [1, D], [0, 8]])
                    nc.gpsimd.tensor_mul(k2b[:, h, :, 0:128], kk0, kk1)
                    kk2 = _ap(kb[:], kb[:, h, c, 9:10], [[FP, 4], [1, 8], [0, 8]])
                    kk3 = _ap(khh[:], khh[:, h, c, 0:1], [[8, 4], [0, 8], [1, 8]])
                    nc.gpsimd.tensor_mul(k2b[:, h, :, 128:192], kk2, kk3)
                    nc.sync.dma_start_transpose(q2tx[:, h], q2b[:, h])
                cur_k2b, cur_q2tx = k2b, q2tx
            k2 = cur_k2b[:, :, c % 4]

            # --- PE: intra scores into 2-chunk PSUM group ---
            if c % 2 == 0:
                stp2 = ps_stp.tile([128, 2, HPC, 128], dt, tag="stp2")
                cur_stp2 = stp2
            stp = cur_stp2[:, c % 2]
            g, p0 = c // 4, 32 * (c % 4)
            for h in range(HPC):
                nc.tensor.matmul(stp[:, h, :], ktp[p0 : p0 + D1, h, g, :],
                                 atp[p0 : p0 + D1, h, g, :],
                                 start=True, stop=True, skip_group_check=True,
                                 tile_position=(p0, 0))

            sq = sb.tile([128, HPC, 128], bt, tag="sq")
            nc.scalar.activation(sq[:], stp[:], Act.Square, scale=RT2I)
            pt = sb.tile([128, HPC, 128], bt, tag="pt")
            mask_bc = _ap(mask[:], mask[:], [[0, HPC], [1, 128]])
            nc.vector.tensor_mul(pt[:], sq[:], mask_bc)
            cdata[c] = (k2, cur_q2tx, pt)

        b = c - LAG
        if b >= 0:
            k2_b, q2tx_b, pt_b = cdata.pop(b)
            gq = 2 * (b % 4)
            # --- PE: readout matmuls -> num PSUM (8-chunk group tile) ---
            if b % 8 == 0:
                num8 = ps_num.tile([128, 8, HPC, D1], dt, tag="num8")
                cur_num8 = num8
            num = cur_num8[:, b % 8]
            for h in range(HPC):
                mms = []
                mms.append(nc.tensor.matmul(num[:, h, :], pt_b[:, h, :],
                                            vb[:, h, b, :], start=True,
                                            stop=False))
                mms.append(nc.tensor.matmul(num[:, h, :], trih[:],
                                            vb[:, h, b, :], start=False,
                                            stop=(b == 0)))
                if b > 0:
                    mms.append(nc.tensor.matmul(num[:, h, :],
                                                q2tx_b[:, h, gq, :],
                                                snap_prev[:, h, 0, :],
                                                start=False, stop=False))
                    mms.append(nc.tensor.matmul(num[:, h, :],
                                                q2tx_b[0:64, h, gq + 1, :],
                                                snap_prev[0:64, h, 1, :],
                                                start=False, stop=False))
                    gb, pb = b // 4, 32 * (b % 4)
                    mms.append(nc.tensor.matmul(num[:, h, :],
                                                atp[pb : pb + D1, h, gb, :],
                                                snap_prev[pb : pb + D1, h, 2, :],
                                                start=False, stop=True,
                                                tile_position=(pb, 0)))
                for m0, m1 in zip(mms, mms[1:]):
                    add_dep_helper(m1.ins, m0.ins, reason="num accum order")

            # --- PE: state update (after previous snapshot read) ---
            umms_h = [[], []]
            for h in range(HPC) if b < NCH - 1 else []:
                umms = umms_h[h]
                umms.append(nc.tensor.matmul(st[:, h, 0, :],
                                             k2_b[:, h, 0:128], vb[:, h, b, :],
                                             start=False, stop=False,
                                             skip_group_check=True))
                umms.append(nc.tensor.matmul(st[0:64, h, 1, :],
                                             k2_b[:, h, 128:192], vb[:, h, b, :],
                                             start=False, stop=False,
                                             skip_group_check=True))
                for rb in range(4):
                    umms.append(nc.tensor.matmul(st[32 * rb : 32 * rb + D1, h, 2, :],
                                                 kb[:, h, b, 0:D1], vb[:, h, b, :],
                                                 start=False, stop=False,
                                                 skip_group_check=True,
                                                 tile_position=(0, 32 * rb)))
            if prev_snap_op is not None:
                for h in range(HPC):
                    for m in umms_h[h]:
                        for cpx in (prev_snap_op if len(prev_snap_op) == 1
                                    else prev_snap_op[h : h + 1]):
                            add_dep_helper(m.ins, cpx.ins,
                                           reason="state WAR after snapshot")

            # --- Pool: snapshot state; tot copy ---
            if b < NCH - 1:
                snap = snapp.tile([128, HPC, 3, D1], bt, tag="snap")
                if b < 24:
                    cp = nc.scalar.copy(snap[:], st[:])
                else:
                    cp = nc.vector.tensor_copy(snap[:], st[:])
                for m in umms_h[0] + umms_h[1]:
                    add_dep_helper(cp.ins, m.ins, reason="snap after update")
                prev_snap_op = (cp,)
                snap_prev = snap
            # --- normalize straight from PSUM; 8-chunk groups, except the
            # last group drains in 4-chunk halves to shorten the tail ---
            epi = None
            if b % 8 == 7 and b < 24:
                epi = (b - 7, 8, 0)
            elif b == 27 or b == 31:
                epi = (b - 3, 4, (b % 8) // 4)
            if epi is not None:
                g0, w_, hf = epi
                csb = slice(g0, b + 1)
                nm = cur_num8[:, 4 * hf : 4 * hf + w_] if w_ == 4 else cur_num8[:]
                rec = bulk.tile([128, 8, HPC, 1], dt, tag=f"rec{b}")
                nc.vector.reciprocal(rec[:, 0:w_], nm[:, :, :, D : D + 1])
                rec_bc = _ap(rec[:], rec[:], [[1, HPC], [HPC, w_], [0, D]])
                num_r = _ap(nm[:], nm[:], [[D1, HPC], [HPC * D1, w_], [1, D]])
                nc.vector.tensor_mul(o_sb[:, :, csb], num_r, rec_bc)
                nc.sync.dma_start(o_d[:, :, csb], o_sb[:, :, csb])

    # ---- epilogue (emitted per half from the loop): nothing left here ----


def build_program():
    nc = bacc.Bacc("TRN2", target_bir_lowering=False, debug=False)
    q_d = nc.dram_tensor("q", [128, HPC, NCH, D], dt, kind="ExternalInput")
    k_d = nc.dram_tensor("k", [128, HPC, NCH, D], dt, kind="ExternalInput")
    v_d = nc.dram_tensor("v", [128, HPC, NCH, D], dt, kind="ExternalInput")
    o_d = nc.dram_tensor("out", [128, HPC, NCH, D], dt, kind="ExternalOutput")

    with tile.TileContext(nc) as tc, ExitStack() as ctx:
        constp = ctx.enter_context(tc.tile_pool(name="const", bufs=1))
        bulk = ctx.enter_context(tc.tile_pool(name="bulk", bufs=1))
        sb = ctx.enter_context(tc.tile_pool(name="sb", bufs=20))
        sbb = ctx.enter_context(tc.tile_pool(name="sbb", bufs=6))
        snapp = ctx.enter_context(tc.tile_pool(name="snap", bufs=6))
        ps_stp = ctx.enter_context(tc.tile_pool(name="ps_stp", bufs=4, space="PSUM"))
        ps_num = ctx.enter_context(tc.tile_pool(name="ps_num", bufs=2, space="PSUM"))
        ps_state = ctx.enter_context(tc.tile_pool(name="ps_st", bufs=1, space="PSUM"))
        ps_kt = ctx.enter_context(tc.tile_pool(name="ps_kt", bufs=1, space="PSUM"))

        from concourse.masks import make_identity
        ident = constp.tile([128, 128], bt)
        make_identity(nc, ident)
        trih = constp.tile([128, 128], bt)
        make_upper_triangular(nc, trih, val=0.5, diag=True)
        mask = constp.tile([128, 128], bt)
        make_upper_triangular(nc, mask, val=1.0, diag=True)

        pools = ((ident, trih, mask), bulk, sb, sbb, snapp, ps_stp, ps_num, ps_state, ps_kt)
        _build_core(nc, pools, q_d, k_d, v_d, o_d)

    nc.compile()
    return nc


_NC = None


def _perm_in(x):
    x = x.reshape(HPC, NCH, 128, D)
    return np.ascontiguousarray(np.transpose(x, (2, 0, 1, 3)))


def kernel(q: np.ndarray, k: np.ndarray, v: np.ndarray) -> np.ndarray:
    global _NC
    if _NC is None:
        _NC = build_program()
    q = np.asarray(q, dtype=np.float32).reshape(H, S, D)
    k = np.asarray(k, dtype=np.float32).reshape(H, S, D)
    v = np.asarray(v, dtype=np.float32).reshape(H, S, D)
    in_maps = []
    for i in range(NCORES):
        sl = slice(i * HPC, (i + 1) * HPC)
        in_maps.append({
            "q": _perm_in(q[sl]),
            "k": _perm_in(k[sl]),
            "v": _perm_in(v[sl]),
        })
    res = run_bass_kernel_spmd(_NC, in_maps, core_ids=list(range(NCORES)))
    outs = []
    for i in range(NCORES):
        o = res.results[i]["out"]
        outs.append(np.transpose(o, (1, 2, 0, 3)).reshape(HPC, S, D))
    return np.concatenate(outs, axis=0).reshape(B, H, S, D)

